# revision 12
# baseline (speedup 1.0000x reference)
"""Multi-head attention (B=2, S=2048, D=1024, H=16) on 8 TRN2 NeuronCores.

Sharding: tensor parallel over heads (2 heads/core) for QKV projection +
attention, then an AllToAll of the context (channel-shard -> row-shard),
then row-parallel output projection. Inputs arrive full; sharding happens
host-side in `kernel()`.

Matmuls run in bf16 (fp32r leaves the PE clock-gated cold and costs 1.5
cyc/row; bf16 is 1 cyc/row, warms HAM, and gets fast weight loads).
Softmax statistics stay fp32 in PSUM; 1/sum is computed as exp(-ln(s)) on
the Scalar engine so the Vector engine never blocks the PE pipeline.

The attention q-range is strided across cores so each of the two AllToAll
halves carries a fully-populated buffer, letting collective #1 and the
first half of the output projection overlap the second attention pass.

Self-contained: shapes hardcoded, no sibling imports.
"""

import numpy as np

B, S, D, H = 2, 2048, 1024, 16
NCORES = 8
CH = D // NCORES          # 128 channels (2 heads) per core
HD = D // H               # 64
ROWS = B * S              # 4096
RPC = ROWS // NCORES      # 512 rows per core for the output projection
KO = D // 128             # 8 contraction chunks of 128
QCH = 1024                # q-chunk processed per attention pass
NQ = S // QCH             # 2 passes
KB = S // 128             # 16 key blocks
RH = RPC // NQ            # 256 rows per core per A2A half
SCALE = 1.0 / 32.0        # 1/sqrt(D)

_CACHE = {}


def _build():
    import concourse.mybir as mybir
    import concourse.tile as tile
    from concourse import bacc
    from concourse.masks import make_identity

    BF16 = mybir.dt.bfloat16
    F32 = mybir.dt.float32
    AF = mybir.ActivationFunctionType

    nc = bacc.Bacc("TRN2", target_bir_lowering=False, debug=False, num_devices=NCORES)
    xT = nc.dram_tensor("xT", [D, ROWS], BF16, kind="ExternalInput")
    # weights arrive host-pre-tiled as [128, KO, out] so DMAs are contiguous
    wq = nc.dram_tensor("wq", [128, KO, CH], BF16, kind="ExternalInput")
    wk = nc.dram_tensor("wk", [128, KO, CH], BF16, kind="ExternalInput")
    wv = nc.dram_tensor("wv", [128, KO, CH], BF16, kind="ExternalInput")
    wo = nc.dram_tensor("wo", [128, KO, D], BF16, kind="ExternalInput")
    out = nc.dram_tensor("out", [RPC, D], F32, kind="ExternalOutput")

    with tile.TileContext(nc) as tc:
        with (
            tc.tile_pool(name="const", bufs=1) as cpool,
            tc.tile_pool(name="qkv", bufs=2) as qkvp,
            tc.tile_pool(name="vtr", bufs=2) as vtrp,
            tc.tile_pool(name="xt", bufs=3) as xtp,
            tc.tile_pool(name="exp", bufs=6) as expp,
            tc.tile_pool(name="bc", bufs=2) as bcp,
            tc.tile_pool(name="cs", bufs=2) as csp,
            tc.tile_pool(name="ph2", bufs=2) as ph2p,
            tc.tile_pool(name="osb", bufs=2) as osbp,
            tc.tile_pool(name="ps", bufs=2, space="PSUM") as ps,
            tc.tile_pool(name="dram", bufs=1, space="DRAM") as dram,
        ):
            w_tiles = {}
            for name, t in (("wq", wq), ("wk", wk), ("wv", wv)):
                wt = cpool.tile([128, KO, CH], BF16, tag=name)
                nc.sync.dma_start(wt[:], t[:])
                w_tiles[name] = wt
            ident = cpool.tile([128, 128], BF16, tag="ident")
            make_identity(nc, ident[:])

            a2a_in = [dram.tile([NCORES, CH, RH], BF16, name=f"a2a_in{p}") for p in range(NQ)]
            a2a_out = [dram.tile([NCORES, CH, RH], BF16, name=f"a2a_out{p}") for p in range(NQ)]

            xT_r = xT.ap().rearrange("(ko p) n -> p ko n", p=128)

            def proj_batch(b):
                """QKV projection for batch b -> transposed [ch, seq] tiles."""
                qt = qkvp.tile([128, S], BF16, tag="qt", name=f"qt{b}")
                kt = qkvp.tile([128, S], BF16, tag="kt", name=f"kt{b}")
                vt = qkvp.tile([128, S], BF16, tag="vt", name=f"vt{b}")
                for rb in range(S // 512):
                    r = b * (S // 512) + rb
                    xt = xtp.tile([128, KO, 512], BF16, tag="xt")
                    nc.sync.dma_start(xt[:], xT_r[:, :, r * 512:(r + 1) * 512])
                    for wname, dst in (("wq", qt), ("wk", kt), ("wv", vt)):
                        pj = ps.tile([128, 512], F32, tag="sc")
                        for ko in range(KO):
                            nc.tensor.matmul(
                                pj[:], w_tiles[wname][:, ko, :], xt[:, ko, :],
                                start=(ko == 0), stop=(ko == KO - 1),
                            )
                        nc.vector.tensor_copy(dst[:, rb * 512:(rb + 1) * 512], pj[:])
                # V back to row-major blocks with a fused ones column:
                # vr[:, kb, 0:65] = [V_h0 | 1], vr[:, kb, 65:130] = [V_h1 | 1]
                vr = vtrp.tile([128, KB, 130], BF16, tag="vtr", name=f"vr{b}")
                nc.vector.memset(vr[:, :, 64:65], 1.0)
                nc.vector.memset(vr[:, :, 129:130], 1.0)
                for kb in range(KB):
                    tp = ps.tile([128, 128], BF16, tag="cx")
                    nc.tensor.transpose(tp[:], vt[:, kb * 128:(kb + 1) * 128], ident[:])
                    nc.vector.tensor_copy(vr[:, kb, 0:64], tp[:, 0:64])
                    nc.vector.tensor_copy(vr[:, kb, 65:129], tp[:, 64:128])
                return qt, kt, vr

            def attention_pass(b, p, qt, kt, vr):
                q0 = p * QCH
                ctx_ps = [
                    ps.tile([65, QCH], F32, tag="cx", name=f"ctx_{b}_{p}_{h}")
                    for h in range(2)
                ]
                for kb in range(KB):
                    ex = []
                    for h in range(2):
                        sc = ps.tile([128, QCH], F32, tag="sc")
                        for n in range(QCH // 512):
                            nc.tensor.matmul(
                                sc[:, n * 512:(n + 1) * 512],
                                kt[h * 64:(h + 1) * 64, kb * 128:(kb + 1) * 128],
                                qt[h * 64:(h + 1) * 64, q0 + n * 512:q0 + (n + 1) * 512],
                                start=True, stop=True,
                                tile_position=(h * 64, 0),
                            )
                        e = expp.tile([128, QCH], BF16, tag="exp")
                        nc.scalar.activation(e[:], sc[:], AF.Exp, scale=SCALE)
                        ex.append(e)
                    for h in range(2):
                        for n in range(QCH // 512):
                            nc.tensor.matmul(
                                ctx_ps[h][:, n * 512:(n + 1) * 512],
                                vr[:, kb, h * 65:(h + 1) * 65],
                                ex[h][:, n * 512:(n + 1) * 512],
                                start=(kb == 0), stop=(kb == KB - 1),
                            )
                # free both psum slots FIRST (fast DVE copies) so the next
                # pass's matmuls never wait; the slow reciprocal runs on the
                # otherwise-idle VectorE after, overlapped with the next pass.
                cfs = []
                for h in range(2):
                    cf = csp.tile([65, QCH], F32, tag="cf", name=f"cf_{b}_{p}_{h}")
                    nc.vector.tensor_copy(cf[:], ctx_ps[h][:])
                    cfs.append(cf)
                last = (b == 1 and p == NQ - 1)
                for h in range(2):
                    cf = cfs[h]
                    bc = bcp.tile([64, QCH], F32, tag="bc")
                    if last and h == 0:
                        lt = bcp.tile([1, QCH], F32, tag="lt")
                        nc.scalar.activation(lt[:], cf[64:65, :], AF.Ln)
                        nc.scalar.activation(bc[0:1, :], lt[:], AF.Exp, scale=-1.0)
                    else:
                        nc.vector.reciprocal(bc[0:1, :], cf[64:65, :])
                    nc.gpsimd.partition_broadcast(bc[:], bc[0:1, :], channels=64)
                    cs = csp.tile([64, QCH], BF16, tag="cs")
                    nc.vector.tensor_mul(cs[:], cf[0:64, :], bc[:])
                    # scatter into the A2A buffer for this half: q within the
                    # pass decomposes as (v, j, i) -> dst core 4b+j, local row
                    # v*128+i
                    nc.sync.dma_start(
                        a2a_in[p][4 * b:4 * b + 4, h * 64:(h + 1) * 64, :]
                        .rearrange("j c (v i) -> c v j i", i=128),
                        cs[:].rearrange("c (v j i) -> c v j i", v=2, j=4),
                    )

            def phase2_half(p, wo_t):
                ctxg = ph2p.tile([128, KO, RH], BF16, tag="ctxg", name=f"ctxg{p}")
                nc.sync.dma_start(ctxg[:], a2a_out[p][:].rearrange("j q r -> q j r"))
                for rb in range(RH // 128):
                    for nh in range(D // 512):
                        pj = ps.tile([128, 512], F32, tag="cx", name=f"p2_{p}_{rb}_{nh}")
                        for j in range(KO):
                            nc.tensor.matmul(
                                pj[:],
                                ctxg[:, j, rb * 128:(rb + 1) * 128],
                                wo_t[:, j, nh * 512:(nh + 1) * 512],
                                start=(j == 0), stop=(j == KO - 1),
                            )
                        ob = osbp.tile([128, 512], F32, tag="osb")
                        nc.vector.tensor_copy(ob[:], pj[:])
                        nc.sync.dma_start(
                            out.ap()[p * RH + rb * 128:p * RH + (rb + 1) * 128,
                                     nh * 512:(nh + 1) * 512],
                            ob[:],
                        )

            qk0 = proj_batch(0)
            wo_t = cpool.tile([128, KO, D], BF16, tag="wo")
            nc.sync.dma_start(wo_t[:], wo[:])

            attention_pass(0, 0, *qk0)
            qk1 = proj_batch(1)
            attention_pass(1, 0, *qk1)
            qk = [qk0, qk1]
            for p in range(NQ):
                if p > 0:
                    attention_pass(0, p, *qk[0])
                    attention_pass(1, p, *qk[1])
                nc.gpsimd.collective_compute(
                    "AllToAll",
                    mybir.AluOpType.bypass,
                    replica_groups=[list(range(NCORES))],
                    ins=[a2a_in[p].opt()],
                    outs=[a2a_out[p].opt()],
                )
            # both phase-2 halves after the last attention pass: half 0's data
            # has long arrived, so its matmuls fill the wait for collective #1
            for p in range(NQ):
                phase2_half(p, wo_t)
    nc.compile()
    return nc


def _numpy_reference(tensor_in, attention_mask, Wq, Wk, Wv, Wo):
    """Fallback for a non-zero mask (never hit with the spec's zero mask)."""
    x = tensor_in.astype(np.float64)
    q = (x @ Wq.T.astype(np.float64)).reshape(B, S, H, HD).transpose(0, 2, 1, 3)
    k = (x @ Wk.T.astype(np.float64)).reshape(B, S, H, HD).transpose(0, 2, 1, 3)
    v = (x @ Wv.T.astype(np.float64)).reshape(B, S, H, HD).transpose(0, 2, 1, 3)
    scores = np.einsum("bhqd,bhkd->bhqk", q, k) + attention_mask.astype(np.float64)
    scores = scores / np.sqrt(D)
    scores -= scores.max(axis=-1, keepdims=True)
    w = np.exp(scores)
    w /= w.sum(axis=-1, keepdims=True)
    ctx = np.einsum("bhqk,bhkd->bhqd", w, v).transpose(0, 2, 1, 3).reshape(B, S, D)
    return (ctx @ Wo.T.astype(np.float64)).astype(np.float32)


def _pretile(wT: np.ndarray) -> np.ndarray:
    """[D, M] -> [128, KO, M] with row d = ko*128 + p."""
    m = wT.shape[1]
    return np.ascontiguousarray(wT.reshape(KO, 128, m).transpose(1, 0, 2))


def _row_map() -> np.ndarray:
    """global row index handled by (core c, local row lr)."""
    m = np.empty((NCORES, RPC), dtype=np.int64)
    for c in range(NCORES):
        bb, jj = c // 4, c % 4
        for p in range(NQ):
            for rb in range(RH // 128):
                u = 2 * p + rb
                g = bb * S + jj * 128 + 512 * u
                lr = p * RH + rb * 128
                m[c, lr:lr + 128] = np.arange(g, g + 128)
    return m


def _run(inputs, trace=False):
    import ml_dtypes
    from concourse.bass_utils import run_bass_kernel_spmd

    bf16 = ml_dtypes.bfloat16
    tensor_in = np.asarray(inputs["tensor_in"], dtype=np.float32)
    Wq = np.asarray(inputs["Wq"], dtype=np.float32)
    Wk = np.asarray(inputs["Wk"], dtype=np.float32)
    Wv = np.asarray(inputs["Wv"], dtype=np.float32)
    Wo = np.asarray(inputs["Wo"], dtype=np.float32)

    xT = np.ascontiguousarray(tensor_in.reshape(ROWS, D).T).astype(bf16)
    wqT = Wq.T.astype(bf16)
    wkT = Wk.T.astype(bf16)
    wvT = Wv.T.astype(bf16)
    wo_p = _pretile(Wo.T.astype(bf16))

    in_maps = []
    for c in range(NCORES):
        sl = slice(c * CH, (c + 1) * CH)
        in_maps.append({
            "xT": xT,
            "wq": _pretile(wqT[:, sl]),
            "wk": _pretile(wkT[:, sl]),
            "wv": _pretile(wvT[:, sl]),
            "wo": wo_p,
        })

    if "nc" not in _CACHE:
        _CACHE["nc"] = _build()
    res = run_bass_kernel_spmd(
        _CACHE["nc"], in_maps, core_ids=list(range(NCORES)), trace=trace
    )
    rm = _CACHE.setdefault("rm", _row_map())
    full = np.empty((ROWS, D), dtype=np.float32)
    for c in range(NCORES):
        full[rm[c]] = res.results[c]["out"]
    return full.reshape(B, S, D), res


def kernel(**inputs) -> np.ndarray:
    mask = np.asarray(inputs["attention_mask"])
    if mask.any():
        return _numpy_reference(
            np.asarray(inputs["tensor_in"]), mask,
            np.asarray(inputs["Wq"]), np.asarray(inputs["Wk"]),
            np.asarray(inputs["Wv"]), np.asarray(inputs["Wo"]),
        )
    out, _ = _run(inputs, trace=False)
    return out
